# revision 23
# baseline (speedup 1.0000x reference)
"""CBOW hierarchical-softmax loss on 8 Trainium2 NeuronCores.

Strategy (collective-free): the node-embedding table (the big one, 400MB) is
row-sharded 8 ways — vocab-parallel, as hinted — while the context table and
the tiny [17,512]x[512] work run replicated on every core.  Each core gathers
the 10 context rows from its full context table, computes h*10 and the full
17 dot products, but only the node rows it owns are gathered from its shard
(host pre-localizes the indices; unowned ones are clamped to row 0).  A
host-provided 0/1 ownership mask weights the final log-loss reduction, so
each path bit is counted by exactly one core, and the host just sums the 8
partial scalars.  No cross-core communication: the NRT collective barrier +
mesh AllReduce (~60us for 68 bytes) is avoided entirely.

Toolchain constraint: every TRN2 instruction encodes a single semaphore
wait, so the dataflow is shaped so each instruction depends on work from at
most one other engine/queue, all input DMAs share one SWDGE semaphore, and
the TileContext tail drain is split into single-wait nops.
"""

import sys

for _p in ("/opt/trn_rl_repo",):
    if _p not in sys.path:
        sys.path.insert(0, _p)

import numpy as np

import concourse.bass as bass
import concourse.mybir as mybir
import concourse.tile as tile
import concourse.tile_sem_assignment as _tsa
from concourse.bass_utils import run_bass_kernel_spmd

VOCAB = 100000
EMBED = 512
WINDOW = 10
PATH = 17
EPS = 1e-9
NCORES = 8
NSH = 2 * VOCAB // NCORES  # 25000 node rows per core

# Index data is packed as COLUMNS of a [17, 516] int32 tensor (ctx indices /
# local node indices / code bits / ownership mask / 512 zero words): indirect
# DMA offset APs must start at partition 0 (a partition-32 offset AP wedges
# the device), and engine reads of SBUF slices must start on 32-aligned
# partitions — column slices at partition base 0 satisfy both.  Columns
# 4..515 arrive as zeros and double (bitcast) as the f32 node-row buffer:
# unowned node indices are set out-of-range host-side and skipped by the
# gather's bounds check, so only owned rows move — the rest stay zero.
IDX_META = 4
IDX_COLS = IDX_META + EMBED  # 516
OOB_IDX = 1 << 20
# aux (f32): cols 0..16 of rows 0..9 = all-ones lhsT of the h-broadcast
# matmul; col 17 = ownership-mask lhsT of the loss reduction.  Both matmul
# stationaries then share base partition 0 with their moving operands.
NAUX_COLS = PATH + 1  # 18

_nc_cache = None

_N_PROCS = 27  # Tile's logical processors: 5 engines + 5 seqs + CC + 8 SW + 8 HW DMA

_ORIG_DRAIN_AND_BARRIER = tile.TileContext._drain_and_barrier


def _split_drain_and_barrier(self, tick_clock, wait_clock):
    """TileContext tail-drain replacement: the stock drain carries one wait per
    live semaphore, but this toolchain's codegen only encodes a single wait
    per instruction.  Emit one single-wait SP nop per live semaphore (threading
    cur_clock so nothing is double-waited), then a waitless drain + the stock
    barrier/teardown."""
    from concourse.vector_clock import ScopedClock, VectorClock

    nc = self.nc
    gc = tick_clock.global_clock
    ticks = [gc.peek_next(i) - 1 for i in range(_N_PROCS)]
    seen = [0] * _N_PROCS
    for p, t in enumerate(ticks):
        if t <= 0:
            continue
        sub = [0] * _N_PROCS
        sub[p] = t
        nop_inst = nc.sync.nop(nofuse=True, hint="drain_wait_split")
        wait_clock.add_sem_waits(
            nop_inst.ins,
            ScopedClock({None: VectorClock(sub)}),
            ScopedClock({None: VectorClock(seen)}),
        )
        seen[p] = t
    drain_inst = nc.sync.drain()
    wait_clock.add_sem_waits(
        drain_inst.ins,
        ScopedClock({None: gc}),
        ScopedClock({None: VectorClock(seen)}),
    )
    nc.all_engine_barrier()
    assert self.sems is not None
    popped = nc._tile_sem_poison_stack.pop()
    assert popped is self._sem_poison
    nc.clear_and_free_semaphores(list(self.sems.allocated().values()))
    nc.all_engine_barrier()


tile.TileContext._drain_and_barrier = _split_drain_and_barrier


def _build():
    global _nc_cache
    if _nc_cache is not None:
        return _nc_cache

    # Cap the DMA-completion semaphore pools: fewer distinct semaphores keeps
    # every instruction within the one-wait budget (same-queue ordering and
    # data dependencies collapse into a single cumulative semaphore wait).
    _tsa.NUM_SWDGE_GLOBAL_SEMS = 2
    _tsa.NUM_HWDGE_SEMS = 2

    nc = bass.Bass(num_devices=NCORES, enable_partition_id=False)
    f32 = mybir.dt.float32
    i32 = mybir.dt.int32
    Alu = mybir.AluOpType
    Act = mybir.ActivationFunctionType

    ctx_emb = nc.dram_tensor("ctx_emb", [VOCAB, EMBED], f32, kind="ExternalInput")
    node_shard = nc.dram_tensor("node_shard", [NSH, EMBED], f32, kind="ExternalInput")
    idx_all = nc.dram_tensor("idx_all", [PATH, IDX_COLS], i32, kind="ExternalInput")
    loss = nc.dram_tensor("loss", [1, 1], f32, kind="ExternalOutput")

    with tile.TileContext(nc) as tc:
        with (
            tc.tile_pool(name="sb", bufs=1) as sb,
            tc.tile_pool(name="ps", bufs=1, space="PSUM") as ps,
        ):
            # idx rides the HW queue (starts during the preamble, before the
            # gpsimd sequencer has even fetched its first instruction); the
            # two gathers get separate SWDGE semaphores so neither waits on
            # the other's completion.
            idx_t = sb.tile([PATH, IDX_COLS], i32)
            nc.sync.dma_start(out=idx_t[:], in_=idx_all[:])

            ctx_rows = sb.tile([WINDOW, EMBED], f32)
            nc.gpsimd.indirect_dma_start(
                out=ctx_rows[:],
                out_offset=None,
                in_=ctx_emb[:],
                in_offset=bass.IndirectOffsetOnAxis(ap=idx_t[:WINDOW, 0:1], axis=0),
            )
            node_rows = idx_t[:, IDX_META:IDX_COLS].bitcast(f32)
            nc.gpsimd.indirect_dma_start(
                out=node_rows,
                out_offset=None,
                in_=node_shard[:],
                in_offset=bass.IndirectOffsetOnAxis(ap=idx_t[:, 1:2], axis=0),
                bounds_check=NSH - 1,
                oob_is_err=False,
            )

            # Early small DVE work (waits only on the idx DMA) so later PE/ACT
            # consumers find these ticks already observed.
            eps_t = sb.tile([PATH, 1], f32)
            nc.vector.memset(eps_t[:], EPS)
            zro_t = sb.tile([PATH, 1], f32)
            nc.vector.memset(zro_t[:], 0.0)
            ones_t = sb.tile([PATH, PATH], f32)
            nc.vector.memset(ones_t[:], 1.0)
            bits_f = sb.tile([PATH, 1], f32)
            nc.vector.tensor_copy(out=bits_f[:], in_=idx_t[:, 2:3])
            mask_f = sb.tile([PATH, 1], f32)
            nc.vector.tensor_copy(out=mask_f[:], in_=idx_t[:, 3:4])
            sgn_t = sb.tile([PATH, 1], f32)  # 2b - 1
            nc.vector.tensor_scalar(
                out=sgn_t[:], in0=bits_f[:], scalar1=2.0, scalar2=-1.0, op0=Alu.mult, op1=Alu.add
            )
            cns_t = sb.tile([PATH, 1], f32)  # 1 - b
            nc.vector.tensor_scalar(
                out=cns_t[:], in0=bits_f[:], scalar1=-1.0, scalar2=1.0, op0=Alu.mult, op1=Alu.add
            )

            # hsum[i, :] = sum_w ctx_sb[w, :] for every i: both matmul
            # operands are DVE-produced, one wait.
            ctx_sb = sb.tile([WINDOW, EMBED], f32)
            nc.vector.tensor_copy(out=ctx_sb[:], in_=ctx_rows[:])
            hsum = ps.tile([PATH, EMBED], f32, space="PSUM")
            nc.tensor.matmul(
                out=hsum[:], lhsT=ones_t[:WINDOW, :], rhs=ctx_sb[:], start=True, stop=True
            )

            # Full dot products: s10[p] = sum_d node[p, d] * hsum[p, d].
            hsum_sb = sb.tile([PATH, EMBED], f32)
            nc.vector.tensor_copy(out=hsum_sb[:], in_=hsum[:])
            node_sb = sb.tile([PATH, EMBED], f32)
            nc.vector.tensor_copy(out=node_sb[:], in_=node_rows)
            prod = sb.tile([PATH, EMBED], f32)
            nc.vector.tensor_mul(out=prod[:], in0=node_sb[:], in1=hsum_sb[:])
            s10 = sb.tile([PATH, 1], f32)
            nc.vector.reduce_sum(out=s10[:], in_=prod[:], axis=mybir.AxisListType.X)

            # scores = sigmoid(s10 / 10) computed as 1 / (1 + exp(-x)) so the
            # saturation tail matches IEEE f32 math rather than an ACT table.
            expnx = sb.tile([PATH, 1], f32)
            nc.scalar.activation(out=expnx[:], in_=s10[:], func=Act.Exp, bias=zro_t[:, :1], scale=-1.0 / WINDOW)
            onep = sb.tile([PATH, 1], f32)
            nc.vector.tensor_scalar_add(out=onep[:], in0=expnx[:], scalar1=1.0)
            scores = sb.tile([PATH, 1], f32)
            nc.vector.reciprocal(out=scores[:], in_=onep[:])

            # sadj = bit ? scores : 1 - scores == scores*(2b-1) + (1-b),
            # exact for b in {0,1} (b=0 keeps the single 1-s rounding of ref).
            sadj = sb.tile([PATH, 1], f32)
            nc.vector.scalar_tensor_tensor(
                out=sadj[:], in0=scores[:], scalar=sgn_t[:, :1], in1=cns_t[:], op0=Alu.mult, op1=Alu.add
            )

            # partial loss = sum_p -mask[p] * ln(sadj + EPS): the ownership
            # mask is the stationary of the partition-reduce matmul.
            lp = sb.tile([PATH, 1], f32)
            nc.scalar.activation(out=lp[:], in_=sadj[:], func=Act.Ln, bias=eps_t[:, :1])
            loss_ps = ps.tile([1, 1], f32, space="PSUM")
            nc.tensor.matmul(
                out=loss_ps[:], lhsT=mask_f[:, :1], rhs=lp[:], start=True, stop=True
            )
            out_sb = sb.tile([1, 1], f32)
            nc.scalar.mul(out=out_sb[:], in_=loss_ps[:], mul=-1.0)
            nc.sync.dma_start(out=loss[:], in_=out_sb[:])

    _nc_cache = nc
    return nc


def _shard_inputs(context_idx, path_indices, code_bits, ctx_emb, node_emb):
    ctx_i = np.asarray(context_idx).astype(np.int64).reshape(WINDOW)
    path_i = np.asarray(path_indices).astype(np.int64).reshape(PATH)
    bits_i = np.asarray(code_bits).astype(np.int32).reshape(PATH)
    ctx_e = np.ascontiguousarray(np.asarray(ctx_emb, dtype=np.float32))
    node_e = np.asarray(node_emb, dtype=np.float32)

    in_maps = []
    for c in range(NCORES):
        lo = c * NSH
        local = path_i - lo
        owned = (local >= 0) & (local < NSH)
        local = np.where(owned, local, 0)

        idx_all = np.zeros((PATH, IDX_COLS), dtype=np.int32)
        idx_all[:WINDOW, 0] = ctx_i
        idx_all[:, 1] = np.where(owned, local, OOB_IDX)
        idx_all[:, 2] = bits_i
        idx_all[:, 3] = owned.astype(np.int32)

        in_maps.append(
            {
                "ctx_emb": ctx_e,
                "node_shard": node_e[lo : lo + NSH],
                "idx_all": idx_all,
            }
        )
    return in_maps


def _run(inputs, trace=False):
    nc = _build()
    in_maps = _shard_inputs(**inputs)
    res = run_bass_kernel_spmd(nc, in_maps, core_ids=list(range(NCORES)), trace=trace)
    total = np.float32(0.0)
    for r in res.results:
        total += np.asarray(r["loss"], dtype=np.float32).reshape(())
    return np.float32(total).reshape(()), res


def kernel(**inputs):
    out, _ = _run(inputs, trace=False)
    return out


# revision 25
# speedup vs baseline: 1.0481x; 1.0481x over previous
"""CBOW hierarchical-softmax loss on 8 Trainium2 NeuronCores.

Strategy (collective-free): the node-embedding table (the big one, 400MB) is
row-sharded 8 ways — vocab-parallel, as hinted — while the context table and
the tiny [17,512]x[512] work run replicated on every core.  Each core gathers
the 10 context rows from its full context table, computes h*10 and the full
17 dot products, but only the node rows it owns are gathered from its shard
(host pre-localizes the indices; unowned ones are clamped to row 0).  A
host-provided 0/1 ownership mask weights the final log-loss reduction, so
each path bit is counted by exactly one core, and the host just sums the 8
partial scalars.  No cross-core communication: the NRT collective barrier +
mesh AllReduce (~60us for 68 bytes) is avoided entirely.

Toolchain constraint: every TRN2 instruction encodes a single semaphore
wait, so the dataflow is shaped so each instruction depends on work from at
most one other engine/queue, all input DMAs share one SWDGE semaphore, and
the TileContext tail drain is split into single-wait nops.
"""

import sys

for _p in ("/opt/trn_rl_repo",):
    if _p not in sys.path:
        sys.path.insert(0, _p)

import numpy as np

import concourse.bass as bass
import concourse.mybir as mybir
import concourse.tile as tile
import concourse.tile_sem_assignment as _tsa
from concourse.bass_utils import run_bass_kernel_spmd

VOCAB = 100000
EMBED = 512
WINDOW = 10
PATH = 17
EPS = 1e-9
NCORES = 8
NSH = 2 * VOCAB // NCORES  # 25000 node rows per core

# Index data is packed as COLUMNS of a [17, 4] int32 tensor (ctx indices /
# local node indices / code bits / ownership mask): indirect-DMA offset APs
# must start at partition 0 (a partition-32 offset AP wedges the device), and
# engine reads of SBUF slices must start on 32-aligned partitions — column
# slices at partition base 0 satisfy both.  Unowned node indices are set
# out-of-range host-side: the gather's bounds check skips them, so only owned
# rows move and the rest of the node buffer keeps the zeros it was DMA'd
# with (a zeros input riding the second HW queue).
IDX_COLS = 4
OOB_IDX = 1 << 20
# aux (f32): cols 0..16 of rows 0..9 = all-ones lhsT of the h-broadcast
# matmul; col 17 = ownership-mask lhsT of the loss reduction.  Both matmul
# stationaries then share base partition 0 with their moving operands.
NAUX_COLS = PATH + 1  # 18

_nc_cache = None

_N_PROCS = 27  # Tile's logical processors: 5 engines + 5 seqs + CC + 8 SW + 8 HW DMA

_ORIG_DRAIN_AND_BARRIER = tile.TileContext._drain_and_barrier


def _split_drain_and_barrier(self, tick_clock, wait_clock):
    """TileContext tail-drain replacement: the stock drain carries one wait per
    live semaphore, but this toolchain's codegen only encodes a single wait
    per instruction.  Emit one single-wait SP nop per live semaphore (threading
    cur_clock so nothing is double-waited), then a waitless drain + the stock
    barrier/teardown."""
    from concourse.vector_clock import ScopedClock, VectorClock

    nc = self.nc
    gc = tick_clock.global_clock
    ticks = [gc.peek_next(i) - 1 for i in range(_N_PROCS)]
    seen = [0] * _N_PROCS
    for p, t in enumerate(ticks):
        if t <= 0:
            continue
        sub = [0] * _N_PROCS
        sub[p] = t
        nop_inst = nc.sync.nop(nofuse=True, hint="drain_wait_split")
        wait_clock.add_sem_waits(
            nop_inst.ins,
            ScopedClock({None: VectorClock(sub)}),
            ScopedClock({None: VectorClock(seen)}),
        )
        seen[p] = t
    drain_inst = nc.sync.drain()
    wait_clock.add_sem_waits(
        drain_inst.ins,
        ScopedClock({None: gc}),
        ScopedClock({None: VectorClock(seen)}),
    )
    nc.all_engine_barrier()
    assert self.sems is not None
    popped = nc._tile_sem_poison_stack.pop()
    assert popped is self._sem_poison
    nc.clear_and_free_semaphores(list(self.sems.allocated().values()))
    nc.all_engine_barrier()


tile.TileContext._drain_and_barrier = _split_drain_and_barrier


def _build():
    global _nc_cache
    if _nc_cache is not None:
        return _nc_cache

    # Cap the DMA-completion semaphore pools: fewer distinct semaphores keeps
    # every instruction within the one-wait budget (same-queue ordering and
    # data dependencies collapse into a single cumulative semaphore wait).
    _tsa.NUM_SWDGE_GLOBAL_SEMS = 2
    _tsa.NUM_HWDGE_SEMS = 3

    nc = bass.Bass(num_devices=NCORES, enable_partition_id=False)
    f32 = mybir.dt.float32
    i32 = mybir.dt.int32
    Alu = mybir.AluOpType
    Act = mybir.ActivationFunctionType

    ctx_emb = nc.dram_tensor("ctx_emb", [VOCAB, EMBED], f32, kind="ExternalInput")
    node_shard = nc.dram_tensor("node_shard", [NSH, EMBED], f32, kind="ExternalInput")
    idx_all = nc.dram_tensor("idx_all", [PATH, IDX_COLS], i32, kind="ExternalInput")
    zeros_in = nc.dram_tensor("zeros_in", [PATH, EMBED], f32, kind="ExternalInput")
    loss = nc.dram_tensor("loss", [1, 1], f32, kind="ExternalOutput")

    with tile.TileContext(nc) as tc:
        with (
            tc.tile_pool(name="sb", bufs=1) as sb,
            tc.tile_pool(name="ps", bufs=1, space="PSUM") as ps,
        ):
            # idx rides the HW queue (starts during the preamble, before the
            # gpsimd sequencer has even fetched its first instruction); the
            # two gathers get separate SWDGE semaphores so neither waits on
            # the other's completion.
            idx_t = sb.tile([PATH, IDX_COLS], i32)
            nc.sync.dma_start(out=idx_t[:], in_=idx_all[:])
            node_rows = sb.tile([PATH, EMBED], f32)
            nc.sync.dma_start(out=node_rows[:], in_=zeros_in[:])

            ctx_rows = sb.tile([WINDOW, EMBED], f32)
            nc.gpsimd.indirect_dma_start(
                out=ctx_rows[:],
                out_offset=None,
                in_=ctx_emb[:],
                in_offset=bass.IndirectOffsetOnAxis(ap=idx_t[:WINDOW, 0:1], axis=0),
            )
            nc.gpsimd.indirect_dma_start(
                out=node_rows[:],
                out_offset=None,
                in_=node_shard[:],
                in_offset=bass.IndirectOffsetOnAxis(ap=idx_t[:, 1:2], axis=0),
                bounds_check=NSH - 1,
                oob_is_err=False,
            )

            # Early small DVE work (waits only on the idx DMA) so later PE/ACT
            # consumers find these ticks already observed.
            eps_t = sb.tile([PATH, 1], f32)
            nc.vector.memset(eps_t[:], EPS)
            zro_t = sb.tile([PATH, 1], f32)
            nc.vector.tensor_copy(out=zro_t[:], in_=node_rows[:, :1])
            ones_t = sb.tile([PATH, PATH], f32)
            nc.vector.memset(ones_t[:], 1.0)
            bits_f = sb.tile([PATH, 1], f32)
            nc.vector.tensor_copy(out=bits_f[:], in_=idx_t[:, 2:3])
            mask_f = sb.tile([PATH, 1], f32)
            nc.vector.tensor_copy(out=mask_f[:], in_=idx_t[:, 3:4])
            sgn_t = sb.tile([PATH, 1], f32)  # 2b - 1
            nc.vector.tensor_scalar(
                out=sgn_t[:], in0=bits_f[:], scalar1=2.0, scalar2=-1.0, op0=Alu.mult, op1=Alu.add
            )
            cns_t = sb.tile([PATH, 1], f32)  # 1 - b
            nc.vector.tensor_scalar(
                out=cns_t[:], in0=bits_f[:], scalar1=-1.0, scalar2=1.0, op0=Alu.mult, op1=Alu.add
            )

            # hsum[i, :] = sum_w ctx_sb[w, :] for every i: both matmul
            # operands are DVE-produced, one wait.
            ctx_sb = sb.tile([WINDOW, EMBED], f32)
            nc.vector.tensor_copy(out=ctx_sb[:], in_=ctx_rows[:])
            hsum = ps.tile([PATH, EMBED], f32, space="PSUM")
            nc.tensor.matmul(
                out=hsum[:], lhsT=ones_t[:WINDOW, :], rhs=ctx_sb[:], start=True, stop=True
            )

            # Full dot products: s10[p] = sum_d node[p, d] * hsum[p, d].
            hsum_sb = sb.tile([PATH, EMBED], f32)
            nc.vector.tensor_copy(out=hsum_sb[:], in_=hsum[:])
            node_sb = sb.tile([PATH, EMBED], f32)
            nc.vector.tensor_copy(out=node_sb[:], in_=node_rows[:])
            prod = sb.tile([PATH, EMBED], f32)
            nc.vector.tensor_mul(out=prod[:], in0=node_sb[:], in1=hsum_sb[:])
            s10 = sb.tile([PATH, 1], f32)
            nc.vector.reduce_sum(out=s10[:], in_=prod[:], axis=mybir.AxisListType.X)

            # scores = sigmoid(s10 / 10) computed as 1 / (1 + exp(-x)) so the
            # saturation tail matches IEEE f32 math rather than an ACT table.
            expnx = sb.tile([PATH, 1], f32)
            nc.scalar.activation(out=expnx[:], in_=s10[:], func=Act.Exp, bias=zro_t[:, :1], scale=-1.0 / WINDOW)
            onep = sb.tile([PATH, 1], f32)
            nc.vector.tensor_scalar_add(out=onep[:], in0=expnx[:], scalar1=1.0)
            scores = sb.tile([PATH, 1], f32)
            nc.vector.reciprocal(out=scores[:], in_=onep[:])

            # sadj = bit ? scores : 1 - scores == scores*(2b-1) + (1-b),
            # exact for b in {0,1} (b=0 keeps the single 1-s rounding of ref).
            sadj = sb.tile([PATH, 1], f32)
            nc.vector.scalar_tensor_tensor(
                out=sadj[:], in0=scores[:], scalar=sgn_t[:, :1], in1=cns_t[:], op0=Alu.mult, op1=Alu.add
            )

            # partial loss = sum_p -mask[p] * ln(sadj + EPS): the ownership
            # mask is the stationary of the partition-reduce matmul.
            lp = sb.tile([PATH, 1], f32)
            nc.scalar.activation(out=lp[:], in_=sadj[:], func=Act.Ln, bias=eps_t[:, :1])
            loss_ps = ps.tile([1, 1], f32, space="PSUM")
            nc.tensor.matmul(
                out=loss_ps[:], lhsT=mask_f[:, :1], rhs=lp[:], start=True, stop=True
            )
            out_sb = sb.tile([1, 1], f32)
            nc.scalar.mul(out=out_sb[:], in_=loss_ps[:], mul=-1.0)
            nc.sync.dma_start(out=loss[:], in_=out_sb[:])

    _nc_cache = nc
    return nc


_ZEROS = np.zeros((PATH, EMBED), dtype=np.float32)


def _shard_inputs(context_idx, path_indices, code_bits, ctx_emb, node_emb):
    ctx_i = np.asarray(context_idx).astype(np.int64).reshape(WINDOW)
    path_i = np.asarray(path_indices).astype(np.int64).reshape(PATH)
    bits_i = np.asarray(code_bits).astype(np.int32).reshape(PATH)
    ctx_e = np.ascontiguousarray(np.asarray(ctx_emb, dtype=np.float32))
    node_e = np.asarray(node_emb, dtype=np.float32)

    in_maps = []
    for c in range(NCORES):
        lo = c * NSH
        local = path_i - lo
        owned = (local >= 0) & (local < NSH)
        local = np.where(owned, local, 0)

        idx_all = np.zeros((PATH, IDX_COLS), dtype=np.int32)
        idx_all[:WINDOW, 0] = ctx_i
        idx_all[:, 1] = np.where(owned, local, OOB_IDX)
        idx_all[:, 2] = bits_i
        idx_all[:, 3] = owned.astype(np.int32)

        in_maps.append(
            {
                "ctx_emb": ctx_e,
                "node_shard": node_e[lo : lo + NSH],
                "idx_all": idx_all,
                "zeros_in": _ZEROS,
            }
        )
    return in_maps


def _run(inputs, trace=False):
    nc = _build()
    in_maps = _shard_inputs(**inputs)
    res = run_bass_kernel_spmd(nc, in_maps, core_ids=list(range(NCORES)), trace=trace)
    total = np.float32(0.0)
    for r in res.results:
        total += np.asarray(r["loss"], dtype=np.float32).reshape(())
    return np.float32(total).reshape(()), res


def kernel(**inputs):
    out, _ = _run(inputs, trace=False)
    return out
